# revision 1
# baseline (speedup 1.0000x reference)
"""Trainium2 Bass kernel for nn_SubspaceLinopFactory (subspace NUDFT forward op).

Math (reference):
  s[a,c,h,w] = x[a,h,w] * mps[c,h,w]
  E[r,k,(h,w)] = exp(-i*(trj[r,0,k]*gy[h] + trj[r,1,k]*gx[w]))   (separable)
  y[a,r,c,k] = sum_hw E * s
  z[r,t,c,k] = sum_a phi[a,t] * y[a,r,c,k] * sqrt_dcf[r,k]
  out[t,c,k] = z[subsamp_idx[t], t, c, k]

Sharding: trajectory r -> core r (R == 8 == n_cores). Each core computes
z[t,c,k] for all t with its own r; host gathers rows where subsamp_idx[t]==r.

Device pipeline per core (separable NUDFT, fp16 matmul operands / f32 accum):
  - trig tables per k-chunk: host stages packed phase inputs in "turns"
    ([sin|cos] halves; the cos half pre-shifted by a quarter turn), ScalarE
    Copy applies the per-partition gy/2pi scale, VectorE int32-cast roundtrip
    gives frac = m-round(m) in [-.5,.5], ScalarE Sin(2pi*frac) -> fp16 tables.
  - stage 1 (TensorE, fp16): P[(a,c,h),k] = sum_w sT[w,ach]*(dcf*cos_x)[w,k],
    Q likewise with sin_x. 6 m-tiles x 512-wide k-chunks, PSUM f32.
  - ScalarE casts P,Q PSUM->SBUF fp16; VectorE products A=cy*P, B=sy*Q,
    C=cy*Q, D=sy*P (fp16 2x mode).
  - h-reduction (TensorE): +-1 selector matmuls contract (ac,h) partitions:
    y_re[ac,k] = sum_h A-B, y_im = -(C+D), PSUM-accumulated over m-tiles.
  - phi expansion (TensorE): z[(t,c),k] = phiT.T @ y  (rows = t*4+c = 128).
  - z_re, z_im [128,1024] f32 -> host gathers into [T,C,K] complex64.
"""
import numpy as np

A, T, C, R, D, K, H, W = 3, 32, 4, 8, 2, 1024, 64, 64
N_CORES = 8
ACH = A * C * H          # 768
MT = ACH // 128          # 6 m-tiles
KC = 512                 # k-chunk (one PSUM bank of f32)
NKC = K // KC            # 2

_CACHE = {}


def _build_nc():
    import concourse.bacc as bacc
    import concourse.tile as tile
    import concourse.mybir as mybir

    AF = mybir.ActivationFunctionType
    OP = mybir.AluOpType
    F32 = mybir.dt.float32
    F16 = mybir.dt.float16
    I32 = mybir.dt.int32
    TWO_PI = float(2 * np.pi)

    nc = bacc.Bacc(None, target_bir_lowering=False)

    # batched inputs: big64 = [txr2 | dcf2 | xr | mr] on 64 partitions,
    # big128 = [tyr2 | pp] on 128, sel = [selp | selm] fp16, phit fp16.
    W64 = 2 * K + 2 * K + ACH + ACH  # 5632
    d_b64 = nc.dram_tensor("b64", [64, W64], F32, kind="ExternalInput")
    d_b128 = nc.dram_tensor("b128", [128, 2 * K + 2], F32, kind="ExternalInput")
    d_sel = nc.dram_tensor("sel", [128, 24 * MT], F16, kind="ExternalInput")
    d_phit = nc.dram_tensor("phit", [12, 128], F16, kind="ExternalInput")
    d_zre = nc.dram_tensor("zre", [128, K], F32, kind="ExternalOutput")
    d_zim = nc.dram_tensor("zim", [128, K], F32, kind="ExternalOutput")

    with tile.TileContext(nc) as tc:
        with (
            tc.tile_pool(name="cst", bufs=1) as cst,
            tc.tile_pool(name="tabw", bufs=2) as tabw,
            tc.tile_pool(name="tbl", bufs=2) as tblp,
            tc.tile_pool(name="work", bufs=3) as work,
            tc.tile_pool(name="psA", bufs=2, space="PSUM") as psA,
            tc.tile_pool(name="psY", bufs=1, space="PSUM") as psY,
            tc.tile_pool(name="psZ", bufs=1, space="PSUM") as psZ,
        ):
            b64 = cst.tile([64, W64], F32)
            b128 = cst.tile([128, 2 * K + 2], F32)
            sel = cst.tile([128, 24 * MT], F16)
            phit = cst.tile([12, 128], F16)
            nc.sync.dma_start(b64[:], d_b64[:])
            nc.sync.dma_start(b128[:], d_b128[:])
            nc.sync.dma_start(sel[:], d_sel[:])
            nc.sync.dma_start(phit[:], d_phit[:])

            txr2 = b64[:, 0:2 * K].rearrange("p (s k) -> p s k", s=2)
            dcf2 = b64[:, 2 * K:4 * K].rearrange("p (s k) -> p s k", s=2)
            xr = b64[:, 4 * K:4 * K + ACH]
            mr = b64[:, 4 * K + ACH:4 * K + 2 * ACH]
            tyr2 = b128[:, 0:2 * K].rearrange("p (s k) -> p s k", s=2)
            ppy = b128[:, 2 * K:2 * K + 1]
            ppx = b128[:64, 2 * K + 1:2 * K + 2]

            # sT = x_rep * mps_rep  -> fp16 [64, ACH]
            sT = cst.tile([64, ACH], F16)
            nc.vector.tensor_tensor(sT[:], xr[:], mr[:], OP.mult)

            selp = sel[:, 0:12 * MT]
            selm = sel[:, 12 * MT:24 * MT]

            zout_re = cst.tile([128, K], F32)
            zout_im = cst.tile([128, K], F32)

            def trig_chunk(src, scale_ap, P, kc, name, out_dt):
                """[P, 2, KC] fp16 table chunk: [:,0,:]=sin, [:,1,:]=cos."""
                ks = slice(kc * KC, (kc + 1) * KC)
                m = tabw.tile([P, 2, KC], F32, tag=f"m{name}")
                nc.scalar.activation(m[:], src[:, :, ks], AF.Copy, scale=scale_ap)
                mi = tabw.tile([P, 2, KC], I32, tag=f"mi{name}")
                nc.vector.tensor_copy(mi[:], m[:])
                mf = tabw.tile([P, 2, KC], F32, tag=f"mf{name}")
                nc.vector.tensor_copy(mf[:], mi[:])
                fr = tabw.tile([P, 2, KC], F32, tag=f"fr{name}")
                nc.vector.tensor_tensor(fr[:], m[:], mf[:], OP.subtract)
                o = tblp.tile([P, 2, KC], out_dt, tag=f"tbl{name}")
                nc.scalar.activation(o[:], fr[:], AF.Sin, scale=TWO_PI)
                return o

            for kc in range(NKC):
                ks = slice(kc * KC, (kc + 1) * KC)
                xt = trig_chunk(txr2, ppx, 64, kc, "x", F32)
                xtd = tblp.tile([64, 2, KC], F16, tag="xtd")
                nc.vector.tensor_tensor(xtd[:], xt[:], dcf2[:, :, ks], OP.mult)
                yt = trig_chunk(tyr2, ppy, 128, kc, "y", F16)

                yre = psY.tile([12, KC], F32, tag="yre")
                yim = psY.tile([12, KC], F32, tag="yim")
                for j in range(MT):
                    js = slice(j * 128, (j + 1) * 128)
                    p_ps = psA.tile([128, KC], F32, tag="p")
                    q_ps = psA.tile([128, KC], F32, tag="q")
                    nc.tensor.matmul(p_ps[:], sT[:, js], xtd[:, 1, :],
                                     start=True, stop=True)
                    nc.tensor.matmul(q_ps[:], sT[:, js], xtd[:, 0, :],
                                     start=True, stop=True)
                    pc = work.tile([128, KC], F16, tag="pc")
                    qc = work.tile([128, KC], F16, tag="qc")
                    nc.scalar.copy(pc[:], p_ps[:])
                    nc.scalar.copy(qc[:], q_ps[:])
                    prodA = work.tile([128, KC], F16, tag="A")
                    prodB = work.tile([128, KC], F16, tag="B")
                    prodC = work.tile([128, KC], F16, tag="C")
                    prodD = work.tile([128, KC], F16, tag="D")
                    nc.vector.tensor_tensor(prodA[:], pc[:], yt[:, 1, :], OP.mult)
                    nc.vector.tensor_tensor(prodB[:], qc[:], yt[:, 0, :], OP.mult)
                    nc.vector.tensor_tensor(prodC[:], qc[:], yt[:, 1, :], OP.mult)
                    nc.vector.tensor_tensor(prodD[:], pc[:], yt[:, 0, :], OP.mult)
                    sj = slice(j * 12, (j + 1) * 12)
                    nc.tensor.matmul(yre[:], selp[:, sj], prodA[:],
                                     start=(j == 0), stop=False,
                                     skip_group_check=True)
                    nc.tensor.matmul(yre[:], selm[:, sj], prodB[:],
                                     start=False, stop=(j == MT - 1),
                                     skip_group_check=True)
                    nc.tensor.matmul(yim[:], selm[:, sj], prodC[:],
                                     start=(j == 0), stop=False,
                                     skip_group_check=True)
                    nc.tensor.matmul(yim[:], selm[:, sj], prodD[:],
                                     start=False, stop=(j == MT - 1),
                                     skip_group_check=True)
                yre_sb = work.tile([12, KC], F16, tag="yre_sb")
                yim_sb = work.tile([12, KC], F16, tag="yim_sb")
                nc.scalar.copy(yre_sb[:], yre[:])
                nc.scalar.copy(yim_sb[:], yim[:])
                zre_ps = psZ.tile([128, KC], F32, tag="zre")
                zim_ps = psZ.tile([128, KC], F32, tag="zim")
                nc.tensor.matmul(zre_ps[:], phit[:], yre_sb[:], start=True, stop=True)
                nc.tensor.matmul(zim_ps[:], phit[:], yim_sb[:], start=True, stop=True)
                nc.scalar.copy(zout_re[:, ks], zre_ps[:])
                nc.scalar.copy(zout_im[:, ks], zim_ps[:])

            nc.gpsimd.dma_start(d_zre[:], zout_re[:])
            nc.gpsimd.dma_start(d_zim[:], zout_im[:])

    nc.finalize()
    return nc


def _get_nc():
    if "nc" not in _CACHE:
        _CACHE["nc"] = _build_nc()
    return _CACHE["nc"]


def _stage_inputs(x, trj, phi, mps, sqrt_dcf):
    """Per-core input maps. Host staging = layout/replication + tiny
    index/scale constants (phase inputs staged in 'turns' with the cos half
    pre-shifted a quarter turn; gy==0 rows use scale=1 with constant input)."""
    f32, f16 = np.float32, np.float16
    gy = np.arange(H, dtype=np.float64) - H // 2
    inv2pi = 1.0 / (2 * np.pi)

    # per-partition scales (col 0: y for 128 rows; col 1: x for 64 rows)
    sc_y = np.where(gy == 0, 1.0, gy * inv2pi)
    pp = np.zeros((128, 2), np.float64)
    pp[:, 0] = np.concatenate([sc_y, sc_y])
    pp[:64, 1] = sc_y

    # cos-half shift: ty + pi/(2*gy) so m_cos = m_sin + 1/4 turn
    with np.errstate(divide="ignore"):
        shift = np.where(gy == 0, 0.0, np.pi / (2 * gy))

    def packed_phase(tv, P):
        """[P, 2, K]: [:,0,:]=tv (sin), [:,1,:]=tv+shift (cos); gy==0 rows
        get constant 0 / 0.25 (scale is 1 there)."""
        g = np.tile(shift, P // H)
        zero = np.tile(gy == 0, P // H)
        out = np.empty((P, 2, K), np.float64)
        out[:, 0, :] = np.where(zero[:, None], 0.0, tv[None, :])
        out[:, 1, :] = np.where(zero[:, None], 0.25, tv[None, :] + g[:, None])
        return out

    # selectors: block j covers ach rows [j*128,(j+1)*128);
    # partition p -> output column ac = 2*j + p//64
    selp = np.zeros((128, 12 * MT), f16)
    for j in range(MT):
        for p in range(128):
            selp[p, j * 12 + 2 * j + p // 64] = 1.0
    sel = np.concatenate([selp, -selp], axis=1)

    phit = np.zeros((12, 128), f16)
    for a in range(A):
        for c in range(C):
            phit[a * 4 + c, c::4] = phi[a].astype(f16)

    xt = np.ascontiguousarray(x.transpose(2, 0, 1))       # [w, a, h]
    xr = np.broadcast_to(xt[:, :, None, :], (W, A, C, H)).reshape(W, ACH)
    mt = np.ascontiguousarray(mps.transpose(2, 0, 1))     # [w, c, h]
    mr = np.broadcast_to(mt[:, None, :, :], (W, A, C, H)).reshape(W, ACH)

    in_maps = []
    for r in range(N_CORES):
        ty = trj[r, 0, :].astype(np.float64)
        tx = trj[r, 1, :].astype(np.float64)
        b64 = np.empty((64, 5632), f32)
        b64[:, 0:2 * K] = packed_phase(tx, 64).reshape(64, 2 * K)
        b64[:, 2 * K:4 * K] = np.broadcast_to(
            sqrt_dcf[r].astype(f32)[None, None, :], (64, 2, K)).reshape(64, 2 * K)
        b64[:, 4 * K:4 * K + ACH] = xr
        b64[:, 4 * K + ACH:] = mr
        b128 = np.empty((128, 2 * K + 2), f32)
        b128[:, 0:2 * K] = packed_phase(ty, 128).reshape(128, 2 * K)
        b128[:, 2 * K:] = pp
        in_maps.append({"b64": b64, "b128": b128, "sel": sel, "phit": phit})
    return in_maps


def kernel(x, trj, phi, mps, sqrt_dcf, subsamp_idx, _trace=False):
    from concourse.bass_utils import run_bass_kernel_spmd

    nc = _get_nc()
    in_maps = _stage_inputs(np.asarray(x), np.asarray(trj), np.asarray(phi),
                            np.asarray(mps), np.asarray(sqrt_dcf))
    res = run_bass_kernel_spmd(nc, in_maps, core_ids=list(range(N_CORES)),
                               trace=_trace)
    out = np.empty((T, C, K), dtype=np.complex64)
    idx = np.asarray(subsamp_idx).astype(np.int64)
    for t in range(T):
        r = int(idx[t])
        zre = res.results[r]["zre"]
        zim = res.results[r]["zim"]
        for c in range(C):
            out[t, c, :] = zre[t * 4 + c] + 1j * zim[t * 4 + c]
    if _trace:
        kernel._last_results = res
    return out



# revision 5
# speedup vs baseline: 1.1300x; 1.1300x over previous
"""Trainium2 Bass kernel for nn_SubspaceLinopFactory (subspace NUDFT forward).

Math (reference):
  s[a,c,h,w] = x[a,h,w] * mps[c,h,w]
  E[r,k,(h,w)] = exp(-i*(ty[k]*gy[h] + tx[k]*gx[w]))   (separable, per r)
  y[a,c,k] = sum_hw E * s ;  z[t,c,k] = sum_a phi[a,t]*y*dcf[k]
  out[t,c,k] = z from core r = subsamp_idx[t]

Sharding: trajectory r -> core r (R == 8 == n_cores).

Per-core pipeline (per 512-wide k-chunk):
  - one fused trig table [128,2,KC]: rows 0-63 y-phases, 64-127 x-phases.
    ScalarE Copy(scale=g/2pi per-partition, bias=16.5/16.75 for sin/cos
    halves), VectorE i32-cast, GpSimd mixed-dtype subtract -> frac,
    ScalarE Sin(scale=-2pi) -> fp16 sin/cos in one shot.
  - xtd = x-trig * dcf (VectorE), cos half DMA'd to partitions 0-63.
  - stage 1: row-tiled concurrent matmuls contract w=64: P (rows 0-63)
    and Q (rows 64-127) into one 2-bank PSUM tile; single ScalarE cast
    to fp16.
  - pair-products (broadcast AP): T1 = P x {sy,cy} = {D,A},
    T2 = Q x {sy,cy} = {B,C}   (VectorE, one j-slice on GpSimd).
  - h-reduction matmuls with phi pre-folded into +/-W [128,128] weights
    accumulate z[(t,c),k] re/im directly in PSUM; ScalarE cast to fp16,
    DMA out.
"""
import numpy as np

A, T, C, R, D, K, H, W = 3, 32, 4, 8, 2, 1024, 64, 64
N_CORES = 8
ACH = A * C * H          # 768
MT = ACH // 128          # 6 m-tiles
KC = 512
NKC = K // KC            # 2

_CACHE = {}


def _build_nc():
    import concourse.bacc as bacc
    import concourse.tile as tile
    import concourse.mybir as mybir

    AF = mybir.ActivationFunctionType
    OP = mybir.AluOpType
    F32 = mybir.dt.float32
    F16 = mybir.dt.float16
    I32 = mybir.dt.int32
    TWO_PI = float(2 * np.pi)

    nc = bacc.Bacc(None, target_bir_lowering=False)

    d_phase = nc.dram_tensor("phase", [2, K], F32, kind="ExternalInput")
    d_sc = nc.dram_tensor("sc", [128, 1], F32, kind="ExternalInput")
    d_sxm = nc.dram_tensor("sxm", [64, 2 * ACH], F16, kind="ExternalInput")
    d_dcf = nc.dram_tensor("dcf", [1, K], F16, kind="ExternalInput")
    d_wp = nc.dram_tensor("wp", [128, MT * 128], F16, kind="ExternalInput")
    d_wm = nc.dram_tensor("wm", [128, MT * 128], F16, kind="ExternalInput")
    d_z = nc.dram_tensor("z", [128, 2, K], F16, kind="ExternalOutput")

    with tile.TileContext(nc) as tc:
        with (
            tc.tile_pool(name="cst", bufs=1) as cst,
            tc.tile_pool(name="trig", bufs=2) as trig,
            tc.tile_pool(name="tbl", bufs=2) as tblp,
            tc.tile_pool(name="work", bufs=3) as work,
            tc.tile_pool(name="zo", bufs=2) as zop,
            tc.tile_pool(name="psPQ", bufs=2, space="PSUM") as psPQ,
            tc.tile_pool(name="psZ", bufs=2, space="PSUM") as psZ,
        ):
            sc = cst.tile([128, 1], F32)
            sxm = cst.tile([64, 2 * ACH], F16)
            wp = cst.tile([128, MT, 128], F16)
            wm = cst.tile([128, MT, 128], F16)
            nc.sync.dma_start(sc[:], d_sc[:])
            nc.sync.dma_start(sxm[:], d_sxm[:])
            nc.sync.dma_start(wp[:], d_wp[:])
            nc.sync.dma_start(wm[:], d_wm[:])

            # sT on partitions 0-63, duplicated to 64-127
            sT = cst.tile([128, ACH], F16)
            nc.vector.tensor_tensor(sT[0:64, :], sxm[:, 0:ACH],
                                    sxm[:, ACH:2 * ACH], OP.mult)
            nc.sync.dma_start(sT[64:128, :], sT[0:64, :])

            for kc in range(NKC):
                ks = slice(kc * KC, (kc + 1) * KC)
                # phase rows broadcast: 0-63 <- ty, 64-127 <- tx
                tin = trig.tile([128, KC], F32, tag="tin")
                nc.sync.dma_start(tin[0:64, :],
                                  d_phase[0:1, ks].broadcast_to([64, KC]))
                nc.sync.dma_start(tin[64:128, :],
                                  d_phase[1:2, ks].broadcast_to([64, KC]))
                dcfb = trig.tile([128, KC], F16, tag="dcfb")
                nc.sync.dma_start(dcfb[64:128, :],
                                  d_dcf[0:1, ks].broadcast_to([64, KC]))

                m = trig.tile([128, 2, KC], F32, tag="m")
                nc.scalar.activation(m[:, 0, :], tin[:], AF.Copy,
                                     scale=sc[:], bias=16.5)
                nc.scalar.activation(m[:, 1, :], tin[:], AF.Copy,
                                     scale=sc[:], bias=16.75)
                mi = trig.tile([128, 2, KC], I32, tag="mi")
                nc.vector.tensor_copy(mi[:], m[:])
                fr = trig.tile([128, 2, KC], F32, tag="fr")
                nc.gpsimd.tensor_tensor(fr[:], m[:], mi[:], OP.subtract)
                tbl = tblp.tile([128, 2, KC], F16, tag="tbl")
                nc.scalar.activation(tbl[:], fr[:], AF.Sin, scale=-TWO_PI)

                # x tables (rows 64-127) * dcf; cos half copied to rows 0-63
                xtd = tblp.tile([128, 2, KC], F16, tag="xtd")
                nc.vector.tensor_tensor(
                    xtd[64:128, :, :], tbl[64:128, :, :],
                    dcfb[64:128, None, :].broadcast_to([64, 2, KC]), OP.mult)
                nc.scalar.dma_start(xtd[0:64, 1, :], xtd[64:128, 1, :])
                # y tables duplicated to all 128 partitions
                ytab = tblp.tile([128, 2, KC], F16, tag="ytab")
                nc.sync.dma_start(ytab[0:64, :, :], tbl[0:64, :, :])
                nc.sync.dma_start(ytab[64:128, :, :], tbl[0:64, :, :])

                zps = psZ.tile([128, 2, KC], F32, tag="zps")
                for j in range(MT):
                    js = slice(j * 128, (j + 1) * 128)
                    pq = psPQ.tile([128, 2, KC], F32, tag="pq")
                    nc.tensor.matmul(pq[:, 0, :], sT[0:64, js],
                                     xtd[0:64, 1, :], start=True, stop=True)
                    nc.tensor.matmul(pq[:, 1, :], sT[64:128, js],
                                     xtd[64:128, 0, :], start=True, stop=True)
                    pq16 = work.tile([128, 2, KC], F16, tag="pq16")
                    nc.scalar.copy(pq16[:], pq[:])
                    t1 = work.tile([128, 2, KC], F16, tag="t1")
                    t2 = work.tile([128, 2, KC], F16, tag="t2")
                    eng = nc.gpsimd if j == 3 else nc.vector
                    eng.tensor_tensor(
                        t1[:], pq16[:, 0:1, :].broadcast_to([128, 2, KC]),
                        ytab[:], OP.mult)
                    eng.tensor_tensor(
                        t2[:], pq16[:, 1:2, :].broadcast_to([128, 2, KC]),
                        ytab[:], OP.mult)
                    # zre += Wp@A - Wm@B ; zim += -Wm@C - Wm@D
                    nc.tensor.matmul(zps[:, 0, :], wp[:, j, :], t1[:, 1, :],
                                     start=(j == 0), stop=False,
                                     skip_group_check=True)
                    nc.tensor.matmul(zps[:, 0, :], wm[:, j, :], t2[:, 0, :],
                                     start=False, stop=(j == MT - 1),
                                     skip_group_check=True)
                    nc.tensor.matmul(zps[:, 1, :], wm[:, j, :], t2[:, 1, :],
                                     start=(j == 0), stop=False,
                                     skip_group_check=True)
                    nc.tensor.matmul(zps[:, 1, :], wm[:, j, :], t1[:, 0, :],
                                     start=False, stop=(j == MT - 1),
                                     skip_group_check=True)
                zout = zop.tile([128, 2, KC], F16, tag="zout")
                nc.scalar.copy(zout[:], zps[:])
                nc.gpsimd.dma_start(d_z[:, :, ks], zout[:])

    nc.finalize()
    return nc


def _get_nc():
    if "nc" not in _CACHE:
        _CACHE["nc"] = _build_nc()
    return _CACHE["nc"]


def _stage_inputs(x, trj, phi, mps, sqrt_dcf):
    """Per-core input maps: layout/replication + tiny scale constants."""
    f32, f16 = np.float32, np.float16
    gy = (np.arange(H, dtype=np.float64) - H // 2)
    inv2pi = 1.0 / (2 * np.pi)

    sc = np.empty((128, 1), f32)
    sc[0:64, 0] = gy * inv2pi
    sc[64:128, 0] = gy * inv2pi    # gx == gy grid

    xt = np.ascontiguousarray(x.transpose(2, 0, 1))       # [w, a, h]
    xr = np.broadcast_to(xt[:, :, None, :], (W, A, C, H)).reshape(W, ACH)
    mt = np.ascontiguousarray(mps.transpose(2, 0, 1))     # [w, c, h]
    mr = np.broadcast_to(mt[:, None, :, :], (W, A, C, H)).reshape(W, ACH)
    sxm = np.concatenate([xr, mr], axis=1).astype(f16)

    # phi folded into h-reduction weights: Wp[j][(i,h),(t,c)] = phi[a_i,t]*(c_i==c)
    wp = np.zeros((128, MT, 128), f16)
    for j in range(MT):
        for i in range(2):
            ac = 2 * j + i
            a, c = ac // C, ac % C
            for t in range(T):
                wp[i * 64:(i + 1) * 64, j, t * C + c] = phi[a, t]
    wm = -wp

    in_maps = []
    for r in range(N_CORES):
        in_maps.append({
            "phase": np.ascontiguousarray(trj[r]).astype(f32),
            "sc": sc,
            "sxm": sxm,
            "dcf": sqrt_dcf[r].reshape(1, K).astype(f16),
            "wp": wp.reshape(128, MT * 128),
            "wm": wm.reshape(128, MT * 128),
        })
    return in_maps


def kernel(x, trj, phi, mps, sqrt_dcf, subsamp_idx, _trace=False):
    from concourse.bass_utils import run_bass_kernel_spmd

    nc = _get_nc()
    in_maps = _stage_inputs(np.asarray(x), np.asarray(trj), np.asarray(phi),
                            np.asarray(mps), np.asarray(sqrt_dcf))
    res = run_bass_kernel_spmd(nc, in_maps, core_ids=list(range(N_CORES)),
                               trace=_trace)
    out = np.empty((T, C, K), dtype=np.complex64)
    idx = np.asarray(subsamp_idx).astype(np.int64)
    for t in range(T):
        r = int(idx[t])
        z = res.results[r]["z"].astype(np.float32)
        for c in range(C):
            out[t, c, :] = z[t * 4 + c, 0, :] + 1j * z[t * 4 + c, 1, :]
    if _trace:
        kernel._last_results = res
    return out


# revision 6
# speedup vs baseline: 1.1398x; 1.0087x over previous
"""Trainium2 Bass kernel for nn_SubspaceLinopFactory (subspace NUDFT forward).

Math (reference):
  s[a,c,h,w] = x[a,h,w] * mps[c,h,w]
  E[r,k,(h,w)] = exp(-i*(ty[k]*gy[h] + tx[k]*gx[w]))   (separable, per r)
  y[a,c,k] = sum_hw E * s ;  z[t,c,k] = sum_a phi[a,t]*y*dcf[k]
  out[t,c,k] = z from core r = subsamp_idx[t]

Sharding: trajectory r -> core r (R == 8 == n_cores).

Per-core pipeline (per 512-wide k-chunk):
  - one fused trig table [128,2,KC]: rows 0-63 y-phases, 64-127 x-phases.
    ScalarE Copy(scale=g/2pi per-partition, bias=16.5/16.75 for sin/cos
    halves), VectorE i32-cast, GpSimd mixed-dtype subtract -> frac,
    ScalarE Sin(scale=-2pi) -> fp16 sin/cos in one shot.
  - xtd = x-trig * dcf (VectorE), cos half DMA'd to partitions 0-63.
  - stage 1: row-tiled concurrent matmuls contract w=64: P (rows 0-63)
    and Q (rows 64-127) into one 2-bank PSUM tile; single ScalarE cast
    to fp16.
  - pair-products (broadcast AP): T1 = P x {sy,cy} = {D,A},
    T2 = Q x {sy,cy} = {B,C}   (VectorE, one j-slice on GpSimd).
  - h-reduction matmuls with phi pre-folded into +/-W [128,128] weights
    accumulate z[(t,c),k] re/im directly in PSUM; ScalarE cast to fp16,
    DMA out.
"""
import numpy as np

A, T, C, R, D, K, H, W = 3, 32, 4, 8, 2, 1024, 64, 64
N_CORES = 8
ACH = A * C * H          # 768
MT = ACH // 128          # 6 m-tiles
KC = 512
NKC = K // KC            # 2

_CACHE = {}


def _build_nc():
    import concourse.bacc as bacc
    import concourse.tile as tile
    import concourse.mybir as mybir

    AF = mybir.ActivationFunctionType
    OP = mybir.AluOpType
    F32 = mybir.dt.float32
    F16 = mybir.dt.float16
    I32 = mybir.dt.int32
    TWO_PI = float(2 * np.pi)

    nc = bacc.Bacc(None, target_bir_lowering=False)

    d_phase = nc.dram_tensor("phase", [2, K], F32, kind="ExternalInput")
    d_sc = nc.dram_tensor("sc", [128, 1], F32, kind="ExternalInput")
    d_sxm = nc.dram_tensor("sxm", [64, 2 * ACH], F16, kind="ExternalInput")
    d_dcf = nc.dram_tensor("dcf", [1, K], F16, kind="ExternalInput")
    d_wp = nc.dram_tensor("wp", [128, MT * 128], F16, kind="ExternalInput")
    d_z = nc.dram_tensor("z", [128, 2, K], F16, kind="ExternalOutput")

    with tile.TileContext(nc) as tc:
        with (
            tc.tile_pool(name="cst", bufs=1) as cst,
            tc.tile_pool(name="trig", bufs=2) as trig,
            tc.tile_pool(name="tbl", bufs=2) as tblp,
            tc.tile_pool(name="work", bufs=3) as work,
            tc.tile_pool(name="zo", bufs=2) as zop,
            tc.tile_pool(name="psPQ", bufs=2, space="PSUM") as psPQ,
            tc.tile_pool(name="psZ", bufs=2, space="PSUM") as psZ,
        ):
            sc = cst.tile([128, 1], F32)
            sxm = cst.tile([64, 2 * ACH], F16)
            wp = cst.tile([128, MT, 128], F16)
            wm = cst.tile([128, MT, 128], F16)
            nc.sync.dma_start(sc[:], d_sc[:])
            nc.sync.dma_start(sxm[:], d_sxm[:])
            nc.sync.dma_start(wp[:], d_wp[:])
            nc.vector.tensor_scalar(wm[:], wp[:], -1.0, None, OP.mult)

            # sT on partitions 0-63, duplicated to 64-127
            sT = cst.tile([128, ACH], F16)
            nc.vector.tensor_tensor(sT[0:64, :], sxm[:, 0:ACH],
                                    sxm[:, ACH:2 * ACH], OP.mult)
            nc.sync.dma_start(sT[64:128, :], sT[0:64, :])

            for kc in range(NKC):
                ks = slice(kc * KC, (kc + 1) * KC)
                # phase rows broadcast: 0-63 <- ty, 64-127 <- tx
                tin = trig.tile([128, KC], F32, tag="tin")
                nc.sync.dma_start(tin[0:64, :],
                                  d_phase[0:1, ks].broadcast_to([64, KC]))
                nc.sync.dma_start(tin[64:128, :],
                                  d_phase[1:2, ks].broadcast_to([64, KC]))
                dcfb = trig.tile([128, 2, KC], F16, tag="dcfb")
                nc.sync.dma_start(dcfb[64:128, 0, :],
                                  d_dcf[0:1, ks].broadcast_to([64, KC]))
                nc.sync.dma_start(dcfb[64:128, 1, :],
                                  d_dcf[0:1, ks].broadcast_to([64, KC]))

                m = trig.tile([128, 2, KC], F32, tag="m")
                nc.vector.tensor_scalar(m[:, 0, :], tin[:], sc[:], 16.5,
                                        OP.mult, OP.add)
                nc.vector.tensor_scalar(m[:, 1, :], tin[:], sc[:], 16.75,
                                        OP.mult, OP.add)
                mi = trig.tile([128, 2, KC], I32, tag="mi")
                nc.vector.tensor_copy(mi[:], m[:])
                fr = trig.tile([128, 2, KC], F32, tag="fr")
                nc.gpsimd.tensor_tensor(fr[:], m[:], mi[:], OP.subtract)
                tbl = tblp.tile([128, 2, KC], F16, tag="tbl")
                nc.scalar.activation(tbl[:], fr[:], AF.Sin, scale=-TWO_PI)

                # x tables (rows 64-127) * dcf; cos half copied to rows 0-63
                xtd = tblp.tile([128, 2, KC], F16, tag="xtd")
                nc.vector.tensor_tensor(
                    xtd[64:128, :, :], tbl[64:128, :, :],
                    dcfb[64:128, :, :], OP.mult)
                nc.sync.dma_start(xtd[0:64, 1, :], xtd[64:128, 1, :])
                # y tables duplicated to all 128 partitions
                ytab = tblp.tile([128, 2, KC], F16, tag="ytab")
                nc.sync.dma_start(ytab[0:64, :, :], tbl[0:64, :, :])
                nc.sync.dma_start(ytab[64:128, :, :], tbl[0:64, :, :])

                zps = psZ.tile([128, 2, KC], F32, tag="zps")
                for j in range(MT):
                    js = slice(j * 128, (j + 1) * 128)
                    pq = psPQ.tile([128, 2, KC], F32, tag="pq")
                    nc.tensor.matmul(pq[:, 0, :], sT[0:64, js],
                                     xtd[0:64, 1, :], start=True, stop=True)
                    nc.tensor.matmul(pq[:, 1, :], sT[64:128, js],
                                     xtd[64:128, 0, :], start=True, stop=True)
                    pq16 = work.tile([128, 2, KC], F16, tag="pq16")
                    nc.scalar.copy(pq16[:], pq[:])
                    t1 = work.tile([128, 2, KC], F16, tag="t1")
                    t2 = work.tile([128, 2, KC], F16, tag="t2")
                    eng = nc.gpsimd if j == 1 else nc.vector
                    eng.tensor_tensor(
                        t1[:], pq16[:, 0:1, :].broadcast_to([128, 2, KC]),
                        ytab[:], OP.mult)
                    eng.tensor_tensor(
                        t2[:], pq16[:, 1:2, :].broadcast_to([128, 2, KC]),
                        ytab[:], OP.mult)
                    # zre += Wp@A - Wm@B ; zim += -Wm@C - Wm@D
                    nc.tensor.matmul(zps[:, 0, :], wp[:, j, :], t1[:, 1, :],
                                     start=(j == 0), stop=False,
                                     skip_group_check=True)
                    nc.tensor.matmul(zps[:, 0, :], wm[:, j, :], t2[:, 0, :],
                                     start=False, stop=(j == MT - 1),
                                     skip_group_check=True)
                    nc.tensor.matmul(zps[:, 1, :], wm[:, j, :], t2[:, 1, :],
                                     start=(j == 0), stop=False,
                                     skip_group_check=True)
                    nc.tensor.matmul(zps[:, 1, :], wm[:, j, :], t1[:, 0, :],
                                     start=False, stop=(j == MT - 1),
                                     skip_group_check=True)
                zout = zop.tile([128, 2, KC], F16, tag="zout")
                nc.scalar.copy(zout[:], zps[:])
                nc.gpsimd.dma_start(d_z[:, :, ks], zout[:])

    nc.finalize()
    return nc


def _get_nc():
    if "nc" not in _CACHE:
        _CACHE["nc"] = _build_nc()
    return _CACHE["nc"]


def _stage_inputs(x, trj, phi, mps, sqrt_dcf):
    """Per-core input maps: layout/replication + tiny scale constants."""
    f32, f16 = np.float32, np.float16
    gy = (np.arange(H, dtype=np.float64) - H // 2)
    inv2pi = 1.0 / (2 * np.pi)

    sc = np.empty((128, 1), f32)
    sc[0:64, 0] = gy * inv2pi
    sc[64:128, 0] = gy * inv2pi    # gx == gy grid

    xt = np.ascontiguousarray(x.transpose(2, 0, 1))       # [w, a, h]
    xr = np.broadcast_to(xt[:, :, None, :], (W, A, C, H)).reshape(W, ACH)
    mt = np.ascontiguousarray(mps.transpose(2, 0, 1))     # [w, c, h]
    mr = np.broadcast_to(mt[:, None, :, :], (W, A, C, H)).reshape(W, ACH)
    sxm = np.concatenate([xr, mr], axis=1).astype(f16)

    # phi folded into h-reduction weights: Wp[j][(i,h),(t,c)] = phi[a_i,t]*(c_i==c)
    wp = np.zeros((128, MT, 128), f16)
    for j in range(MT):
        for i in range(2):
            ac = 2 * j + i
            a, c = ac // C, ac % C
            for t in range(T):
                wp[i * 64:(i + 1) * 64, j, t * C + c] = phi[a, t]
    in_maps = []
    for r in range(N_CORES):
        in_maps.append({
            "phase": np.ascontiguousarray(trj[r]).astype(f32),
            "sc": sc,
            "sxm": sxm,
            "dcf": sqrt_dcf[r].reshape(1, K).astype(f16),
            "wp": wp.reshape(128, MT * 128),
        })
    return in_maps


def kernel(x, trj, phi, mps, sqrt_dcf, subsamp_idx, _trace=False):
    from concourse.bass_utils import run_bass_kernel_spmd

    nc = _get_nc()
    in_maps = _stage_inputs(np.asarray(x), np.asarray(trj), np.asarray(phi),
                            np.asarray(mps), np.asarray(sqrt_dcf))
    res = run_bass_kernel_spmd(nc, in_maps, core_ids=list(range(N_CORES)),
                               trace=_trace)
    out = np.empty((T, C, K), dtype=np.complex64)
    idx = np.asarray(subsamp_idx).astype(np.int64)
    for t in range(T):
        r = int(idx[t])
        z = res.results[r]["z"].astype(np.float32)
        for c in range(C):
            out[t, c, :] = z[t * 4 + c, 0, :] + 1j * z[t * 4 + c, 1, :]
    if _trace:
        kernel._last_results = res
    return out


# revision 7
# speedup vs baseline: 1.3135x; 1.1524x over previous
"""Trainium2 Bass kernel for nn_SubspaceLinopFactory (subspace NUDFT forward).

Math (reference):
  s[a,c,h,w] = x[a,h,w] * mps[c,h,w]
  E[r,k,(h,w)] = exp(-i*(ty[k]*gy[h] + tx[k]*gx[w]))   (separable, per r)
  y[a,c,k] = sum_hw E * s ;  z[t,c,k] = sum_a phi[a,t]*y*dcf[k]
  out[t,c,k] = z from core r = subsamp_idx[t]

Sharding: trajectory r -> core r (R == 8 == n_cores).

Per-core pipeline (per 512-wide k-chunk):
  - one fused trig table [128,2,KC]: rows 0-63 y-phases, 64-127 x-phases.
    ScalarE Copy(scale=g/2pi per-partition, bias=16.5/16.75 for sin/cos
    halves), VectorE i32-cast, GpSimd mixed-dtype subtract -> frac,
    ScalarE Sin(scale=-2pi) -> fp16 sin/cos in one shot.
  - xtd = x-trig * dcf (VectorE), cos half DMA'd to partitions 0-63.
  - stage 1: row-tiled concurrent matmuls contract w=64: P (rows 0-63)
    and Q (rows 64-127) into one 2-bank PSUM tile; single ScalarE cast
    to fp16.
  - pair-products (broadcast AP): T1 = P x {sy,cy} = {D,A},
    T2 = Q x {sy,cy} = {B,C}   (VectorE, one j-slice on GpSimd).
  - h-reduction matmuls with phi pre-folded into +/-W [128,128] weights
    accumulate z[(t,c),k] re/im directly in PSUM; ScalarE cast to fp16,
    DMA out.
"""
import numpy as np

A, T, C, R, D, K, H, W = 3, 32, 4, 8, 2, 1024, 64, 64
N_CORES = 8
ACH = A * C * H          # 768
MT = ACH // 128          # 6 m-tiles
KC = 512
NKC = K // KC            # 2

_CACHE = {}


def _build_nc():
    import concourse.bacc as bacc
    import concourse.tile as tile
    import concourse.mybir as mybir

    AF = mybir.ActivationFunctionType
    OP = mybir.AluOpType
    F32 = mybir.dt.float32
    F16 = mybir.dt.float16
    I32 = mybir.dt.int32
    TWO_PI = float(2 * np.pi)

    nc = bacc.Bacc(None, target_bir_lowering=False)

    d_phase = nc.dram_tensor("phase", [2, K], F32, kind="ExternalInput")
    d_sc = nc.dram_tensor("sc", [128, 1], F32, kind="ExternalInput")
    d_sxm = nc.dram_tensor("sxm", [64, 2 * ACH], F16, kind="ExternalInput")
    d_dcf = nc.dram_tensor("dcf", [1, K], F16, kind="ExternalInput")
    d_wp = nc.dram_tensor("wp", [128, MT * 128], F16, kind="ExternalInput")
    d_z = nc.dram_tensor("z", [128, 2, K], F16, kind="ExternalOutput")

    with tile.TileContext(nc) as tc:
        with (
            tc.tile_pool(name="cst", bufs=1) as cst,
            tc.tile_pool(name="trig", bufs=2) as trig,
            tc.tile_pool(name="tbl", bufs=2) as tblp,
            tc.tile_pool(name="work", bufs=3) as work,
            tc.tile_pool(name="zo", bufs=2) as zop,
            tc.tile_pool(name="psPQ", bufs=2, space="PSUM") as psPQ,
            tc.tile_pool(name="psZ", bufs=2, space="PSUM") as psZ,
        ):
            sc = cst.tile([128, 1], F32)
            sxm = cst.tile([64, 2 * ACH], F16)
            wp = cst.tile([128, MT, 128], F16)
            wm = cst.tile([128, MT, 128], F16)
            nc.sync.dma_start(sc[:], d_sc[:])
            nc.gpsimd.dma_start(sxm[:], d_sxm[:])
            nc.gpsimd.dma_start(wp[:], d_wp[:])
            nc.vector.tensor_scalar(wm[:], wp[:], -1.0, None, OP.mult)

            # sT on partitions 0-63, duplicated to 64-127
            sT = cst.tile([128, ACH], F16)
            nc.vector.tensor_tensor(sT[0:64, :], sxm[:, 0:ACH],
                                    sxm[:, ACH:2 * ACH], OP.mult)
            nc.gpsimd.dma_start(sT[64:128, :], sT[0:64, :])

            for kc in range(NKC):
                ks = slice(kc * KC, (kc + 1) * KC)
                # phase rows broadcast: 0-63 <- ty, 64-127 <- tx
                tin = trig.tile([128, KC], F32, tag="tin")
                nc.sync.dma_start(tin[0:64, :],
                                  d_phase[0:1, ks].broadcast_to([64, KC]))
                nc.sync.dma_start(tin[64:128, :],
                                  d_phase[1:2, ks].broadcast_to([64, KC]))
                dcfb = trig.tile([128, 2, KC], F16, tag="dcfb")
                nc.sync.dma_start(dcfb[64:128, 0, :],
                                  d_dcf[0:1, ks].broadcast_to([64, KC]))
                nc.sync.dma_start(dcfb[64:128, 1, :],
                                  d_dcf[0:1, ks].broadcast_to([64, KC]))

                m = trig.tile([128, 2, KC], F32, tag="m")
                nc.vector.tensor_scalar(m[:, 0, :], tin[:], sc[:], 16.5,
                                        OP.mult, OP.add)
                nc.vector.tensor_scalar(m[:, 1, :], tin[:], sc[:], 16.75,
                                        OP.mult, OP.add)
                mi = trig.tile([128, 2, KC], I32, tag="mi")
                nc.vector.tensor_copy(mi[:], m[:])
                fr = trig.tile([128, 2, KC], F32, tag="fr")
                feng = nc.vector if kc == 0 else nc.gpsimd
                feng.tensor_tensor(fr[:], m[:], mi[:], OP.subtract)
                tbl = tblp.tile([128, 2, KC], F16, tag="tbl")
                nc.scalar.activation(tbl[:], fr[:], AF.Sin, scale=-TWO_PI)

                # x tables (rows 64-127) * dcf; cos half copied to rows 0-63
                xtd = tblp.tile([128, 2, KC], F16, tag="xtd")
                nc.vector.tensor_tensor(
                    xtd[64:128, :, :], tbl[64:128, :, :],
                    dcfb[64:128, :, :], OP.mult)
                nc.sync.dma_start(xtd[0:64, 1, :], xtd[64:128, 1, :])
                # y tables duplicated to all 128 partitions
                ytab = tblp.tile([128, 2, KC], F16, tag="ytab")
                nc.sync.dma_start(ytab[0:64, :, :], tbl[0:64, :, :])
                nc.sync.dma_start(ytab[64:128, :, :], tbl[0:64, :, :])

                zps = psZ.tile([128, 2, KC], F32, tag="zps")
                for j in range(MT):
                    js = slice(j * 128, (j + 1) * 128)
                    pq = psPQ.tile([128, 2, KC], F32, tag="pq")
                    nc.tensor.matmul(pq[:, 0, :], sT[0:64, js],
                                     xtd[0:64, 1, :], start=True, stop=True)
                    nc.tensor.matmul(pq[:, 1, :], sT[64:128, js],
                                     xtd[64:128, 0, :], start=True, stop=True)
                    pq16 = work.tile([128, 2, KC], F16, tag="pq16")
                    nc.scalar.copy(pq16[:], pq[:])
                    # merged product: t12[p, i, s, k] = pq16[p, i, k]*ytab[p, s, k]
                    # i: 0=P,1=Q; s: 0=sy,1=cy -> [P*sy|P*cy|Q*sy|Q*cy]=[D|A|B|C]
                    t12 = work.tile([128, 2, 2, KC], F16, tag="t12")
                    nc.vector.tensor_tensor(
                        t12[:],
                        pq16[:, :, None, :].broadcast_to([128, 2, 2, KC]),
                        ytab[:, None, :, :].broadcast_to([128, 2, 2, KC]),
                        OP.mult)
                    # zre += Wp@A - Wm@B ; zim += -Wm@C - Wm@D
                    nc.tensor.matmul(zps[:, 0, :], wp[:, j, :], t12[:, 0, 1, :],
                                     start=(j == 0), stop=False,
                                     skip_group_check=True)
                    nc.tensor.matmul(zps[:, 0, :], wm[:, j, :], t12[:, 1, 0, :],
                                     start=False, stop=(j == MT - 1),
                                     skip_group_check=True)
                    nc.tensor.matmul(zps[:, 1, :], wm[:, j, :], t12[:, 1, 1, :],
                                     start=(j == 0), stop=False,
                                     skip_group_check=True)
                    nc.tensor.matmul(zps[:, 1, :], wm[:, j, :], t12[:, 0, 0, :],
                                     start=False, stop=(j == MT - 1),
                                     skip_group_check=True)
                zout = zop.tile([128, 2, KC], F16, tag="zout")
                nc.scalar.copy(zout[:], zps[:])
                nc.gpsimd.dma_start(d_z[:, :, ks], zout[:])

    nc.finalize()
    return nc


def _get_nc():
    if "nc" not in _CACHE:
        _CACHE["nc"] = _build_nc()
    return _CACHE["nc"]


def _stage_inputs(x, trj, phi, mps, sqrt_dcf):
    """Per-core input maps: layout/replication + tiny scale constants."""
    f32, f16 = np.float32, np.float16
    gy = (np.arange(H, dtype=np.float64) - H // 2)
    inv2pi = 1.0 / (2 * np.pi)

    sc = np.empty((128, 1), f32)
    sc[0:64, 0] = gy * inv2pi
    sc[64:128, 0] = gy * inv2pi    # gx == gy grid

    xt = np.ascontiguousarray(x.transpose(2, 0, 1))       # [w, a, h]
    xr = np.broadcast_to(xt[:, :, None, :], (W, A, C, H)).reshape(W, ACH)
    mt = np.ascontiguousarray(mps.transpose(2, 0, 1))     # [w, c, h]
    mr = np.broadcast_to(mt[:, None, :, :], (W, A, C, H)).reshape(W, ACH)
    sxm = np.concatenate([xr, mr], axis=1).astype(f16)

    # phi folded into h-reduction weights: Wp[j][(i,h),(t,c)] = phi[a_i,t]*(c_i==c)
    wp = np.zeros((128, MT, 128), f16)
    for j in range(MT):
        for i in range(2):
            ac = 2 * j + i
            a, c = ac // C, ac % C
            for t in range(T):
                wp[i * 64:(i + 1) * 64, j, t * C + c] = phi[a, t]
    in_maps = []
    for r in range(N_CORES):
        in_maps.append({
            "phase": np.ascontiguousarray(trj[r]).astype(f32),
            "sc": sc,
            "sxm": sxm,
            "dcf": sqrt_dcf[r].reshape(1, K).astype(f16),
            "wp": wp.reshape(128, MT * 128),
        })
    return in_maps


def kernel(x, trj, phi, mps, sqrt_dcf, subsamp_idx, _trace=False):
    from concourse.bass_utils import run_bass_kernel_spmd

    nc = _get_nc()
    in_maps = _stage_inputs(np.asarray(x), np.asarray(trj), np.asarray(phi),
                            np.asarray(mps), np.asarray(sqrt_dcf))
    res = run_bass_kernel_spmd(nc, in_maps, core_ids=list(range(N_CORES)),
                               trace=_trace)
    out = np.empty((T, C, K), dtype=np.complex64)
    idx = np.asarray(subsamp_idx).astype(np.int64)
    for t in range(T):
        r = int(idx[t])
        z = res.results[r]["z"].astype(np.float32)
        for c in range(C):
            out[t, c, :] = z[t * 4 + c, 0, :] + 1j * z[t * 4 + c, 1, :]
    if _trace:
        kernel._last_results = res
    return out
